# revision 3
# baseline (speedup 1.0000x reference)
"""2-layer GCN (GCNConv -> ReLU -> BN -> GCNConv -> ReLU) on 8 trn2 NeuronCores.

v2 changes over the staged baseline:
  - P1 is SHARDED: each core computes h1 = (x_shard @ W1) * dinv for its own
    12.5k rows only (26 MB of x per core instead of 205 MB), then one
    AllGather (Shared DRAM) replicates the layer-1 table to all cores.
  - Layer-1 self-loop term comes from a direct DMA of the core's own
    contiguous h1s rows instead of an extra gather slot per tile
    (saves 98 indirect-DMA instructions per core).
  - Both gather tables (tab1 via AG1, tab2 via AG2) live in Shared DRAM,
    which benches ~15-20% faster for indirect gathers than Local.

Everything else (degree-ranked round-robin dst sharding, dinv folding into
the tables, BN folded into W2' and c2, per-tile K-padded single-offset
indirect gathers at one index per partition) is as in the baseline.
"""

import numpy as np

import concourse.bass as bass
import concourse.bacc as bacc
import concourse.mybir as mybir
import concourse.tile as tile
from concourse.bass_utils import run_bass_kernel_spmd

F32 = mybir.dt.float32
I32 = mybir.dt.int32

C = 8          # cores
P = 128        # partitions
H = 32         # hidden dim
D = 512        # input dim
BN_EPS = 1e-5


def _plan(n_nodes, edge_index):
    """Host-side graph preprocessing -> per-core index arrays + metadata."""
    src = np.asarray(edge_index[0], dtype=np.int64)
    dst = np.asarray(edge_index[1], dtype=np.int64)

    deg = np.bincount(dst, minlength=n_nodes).astype(np.float32) + 1.0
    dinv = (1.0 / np.sqrt(deg)).astype(np.float32)

    per = n_nodes // C                      # real rows per core
    SH = -(-per // 512) * 512               # shard rows, multiple of 512
    T_real = (per + P - 1) // P
    T_all = SH // P

    # --- dst ownership: ascending-degree rank, round-robin across cores ---
    order = np.argsort(deg, kind="stable")
    owner = np.empty(n_nodes, dtype=np.int64)
    pos = np.empty(n_nodes, dtype=np.int64)
    ranks = np.arange(n_nodes)
    owner[order] = ranks % C
    pos[order] = ranks // C
    assert pos.max() == per - 1

    e_owner = owner[dst]
    e_pos = pos[dst]
    counts = np.zeros((C, per), dtype=np.int64)
    np.add.at(counts, (e_owner, e_pos), 1)

    # per-tile K = max indegree within the 128-dst tile across all cores.
    # Layer 1's self-loop is a direct DMA (no slot); layer 2's likewise.
    K_list = []
    for t in range(T_real):
        lo, hi = t * P, min((t + 1) * P, per)
        K_list.append(int(counts[:, lo:hi].max()))
    totK = sum(K_list)
    offs = np.concatenate([[0], np.cumsum(K_list)]).astype(np.int64)

    # node n lives at shard row pos(n) of core owner(n); global table row
    # row(n) = owner*SH + pos. Row `per` (core 0's first pad row) is zero.
    row = owner * SH + pos
    pad = per

    nodes_by_cp = np.full((C, per), -1, dtype=np.int64)
    nodes_by_cp[owner, pos] = np.arange(n_nodes)

    idx = np.full((C, P, totK), pad, dtype=np.int32)
    eorder = np.lexsort((src, e_pos, e_owner))
    so, sp, ss = e_owner[eorder], e_pos[eorder], src[eorder]
    grp = so * per + sp
    newgrp = np.ones(len(grp), dtype=bool)
    newgrp[1:] = grp[1:] != grp[:-1]
    gstart = np.where(newgrp)[0]
    slot = np.arange(len(grp)) - np.repeat(
        gstart, np.diff(np.concatenate([gstart, [len(grp)]])))
    tt = sp // P
    lane = sp % P
    idx[so, lane, offs[tt] + slot] = row[ss]

    # dinv in sorted-shard order (per core)
    dinv_s = np.zeros((C, P, T_all), dtype=np.float32)
    for c in range(C):
        fulls = np.zeros(SH, np.float32)
        fulls[:per] = dinv[nodes_by_cp[c]]
        dinv_s[c] = fulls.reshape(T_all, P).T

    meta = dict(per=per, SH=SH, T_real=T_real, T_all=T_all,
                K_list=K_list, offs=offs, totK=totK,
                nodes_by_cp=nodes_by_cp, dinv=dinv)
    return idx, dinv_s, meta


def _build_nc(n_nodes, meta, phases=("p1", "ag1", "l1", "ag2", "l2"),
              reps=1, tab_bf16=False):
    phases = set(phases)
    SH, T_real, T_all = meta["SH"], meta["T_real"], meta["T_all"]
    totK = meta["totK"]
    TAB = C * SH
    TD = mybir.dt.bfloat16 if tab_bf16 else F32

    BF16 = mybir.dt.bfloat16
    nc = bacc.Bacc("TRN2", target_bir_lowering=False, debug=False, num_devices=C)
    xT = nc.dram_tensor("xT", [D, SH], BF16, kind="ExternalInput").ap()
    w1 = nc.dram_tensor("w1", [D, H], F32, kind="ExternalInput").ap()
    w2p = nc.dram_tensor("w2p", [H, H], F32, kind="ExternalInput").ap()
    b1r = nc.dram_tensor("b1r", [P, H], F32, kind="ExternalInput").ap()
    b2r = nc.dram_tensor("b2r", [P, H], F32, kind="ExternalInput").ap()
    c2r = nc.dram_tensor("c2r", [P, H], F32, kind="ExternalInput").ap()
    ident = nc.dram_tensor("ident", [P, P], F32, kind="ExternalInput").ap()
    dinvs = nc.dram_tensor("dinvs", [P, T_all], F32, kind="ExternalInput").ap()
    idx1 = nc.dram_tensor("idx1", [P, totK], I32, kind="ExternalInput").ap()
    out = nc.dram_tensor("out", [SH, H], F32, kind="ExternalOutput").ap()

    with tile.TileContext(nc) as tc:
        with (
            tc.tile_pool(name="cst", bufs=1) as cst,
            tc.tile_pool(name="sb", bufs=3) as sb,
            tc.tile_pool(name="gp", bufs=3) as gp,
            tc.tile_pool(name="ps", bufs=2, space="PSUM") as ps,
            tc.tile_pool(name="dram", bufs=1, space="DRAM") as dram,
        ):
            h1s = dram.tile([SH, H], TD)
            h2s = dram.tile([SH, H], TD)
            h2f = dram.tile([SH, H], F32)

            # constants
            w1f = cst.tile([P, 4 * H], F32)
            for f in range(4):
                nc.sync.dma_start(w1f[:, f * H:(f + 1) * H],
                                  w1[f * P:(f + 1) * P, :])
            w1t = cst.tile([P, 4 * H], mybir.dt.bfloat16)
            nc.vector.tensor_copy(w1t[:], w1f[:])
            w2pt = cst.tile([H, H], F32)
            nc.sync.dma_start(w2pt[:], w2p[:, :])
            b1t = cst.tile([P, H], F32)
            nc.sync.dma_start(b1t[:], b1r[:, :])
            b2t = cst.tile([P, H], F32)
            nc.sync.dma_start(b2t[:], b2r[:, :])
            c2t = cst.tile([P, H], F32)
            nc.sync.dma_start(c2t[:], c2r[:, :])
            idt = cst.tile([P, P], F32)
            nc.sync.dma_start(idt[:], ident[:, :])
            dst_ = cst.tile([P, T_all], F32)
            nc.sync.dma_start(dst_[:], dinvs[:, :])
            ix1 = cst.tile([P, totK], I32)
            nc.sync.dma_start(ix1[:], idx1[:, :])
            ztd = cst.tile([P, H], TD)
            nc.vector.memset(ztd[:], 0.0)

            env = dict(locals())
            for _rep in range(reps):
                t1r = dram.tile([TAB, H], TD, addr_space="Shared",
                                tag=f"tab1r{_rep}")
                t2r = dram.tile([TAB, H], TD, addr_space="Shared",
                                tag=f"tab2r{_rep}")
                env["tab1"] = t1r
                env["tab2"] = t2r
                _body(nc, tc, phases, meta, env)

    nc.compile()
    return nc


def _body(nc, tc, phases, meta, env):
    SH, T_real, T_all = meta["SH"], meta["T_real"], meta["T_all"]
    K_list, offs = meta["K_list"], meta["offs"]
    NST = SH // 512
    maxK = max(K_list)
    TD = env["TD"]
    xT = env["xT"]; out = env["out"]
    sb = env["sb"]; gp = env["gp"]; ps = env["ps"]
    h1s = env["h1s"]; h2s = env["h2s"]; h2f = env["h2f"]
    tab1 = env["tab1"]; tab2 = env["tab2"]
    w1t = env["w1t"]; w2pt = env["w2pt"]; b1t = env["b1t"]; b2t = env["b2t"]
    c2t = env["c2t"]; idt = env["idt"]; dst_ = env["dst_"]
    ix1 = env["ix1"]; ix2 = ix1; ztd = env["ztd"]

    # ---- P1 (sharded): own 12.5k rows of h1 = (x @ W1) * dinv ----
    for st in range(NST if "p1" in phases else 0):
        xt = sb.tile([P, 4 * D], mybir.dt.bfloat16, tag="xt")
        for f in range(4):
            nc.sync.dma_start(
                xt[:, f * D:(f + 1) * D],
                xT[f * P:(f + 1) * P, st * 512:(st + 1) * 512])
        for g4 in range(4):
            pp = ps.tile([P, H], F32, tag="p1ps")
            for f in range(4):
                nc.tensor.matmul(
                    pp[:],
                    lhsT=xt[:, f * D + g4 * P: f * D + (g4 + 1) * P],
                    rhs=w1t[:, f * H:(f + 1) * H],
                    start=(f == 0), stop=(f == 3))
            g = st * 4 + g4
            ht = sb.tile([P, H], TD, tag="ht")
            nc.scalar.activation(ht[:], pp[:],
                                 mybir.ActivationFunctionType.Copy,
                                 scale=dst_[:, g:g + 1])
            nc.sync.dma_start(h1s[g * P:(g + 1) * P, :], ht[:])

    # zero pad rows of h2s (tiles >= T_real never written)
    if "p1" in phases:
        for t in range(T_real, T_all):
            nc.sync.dma_start(h2s[t * P:(t + 1) * P, :], ztd[:])

    # ---- AllGather 1: replicate the layer-1 table ----
    if "ag1" in phases:
        nc.gpsimd.collective_compute(
            "AllGather", mybir.AluOpType.bypass,
            replica_groups=[list(range(C))],
            ins=[h1s.opt()], outs=[tab1.opt()])

    # ---- Layer 1 aggregation + epilogue (self via local h1s DMA) ----
    for t in range(T_real if "l1" in phases else 0):
        K = K_list[t]
        sf = sb.tile([P, H], TD, tag="sf1")
        nc.sync.dma_start(sf[:], h1s[t * P:(t + 1) * P, :])
        red = sb.tile([P, H], F32, tag="red")
        if K > 0:
            g = gp.tile([P, maxK * H], TD, tag="g1")
            for j in range(K):
                nc.gpsimd.indirect_dma_start(
                    out=g[:, j * H:(j + 1) * H], out_offset=None,
                    in_=tab1[:],
                    in_offset=bass.IndirectOffsetOnAxis(
                        ap=ix1[:, offs[t] + j: offs[t] + j + 1], axis=0))
            nc.vector.reduce_sum(
                out=red[:],
                in_=g[:, :K * H].rearrange("p (j f) -> p f j", f=H),
                axis=mybir.AxisListType.X)
            nc.vector.tensor_add(red[:], red[:], sf[:])
        else:
            nc.vector.tensor_copy(red[:], sf[:])
        nc.vector.tensor_scalar_mul(red[:], red[:], dst_[:, t:t + 1])
        nc.vector.tensor_add(red[:], red[:], b1t[:])
        nc.vector.tensor_scalar_max(red[:], red[:], 0.0)
        pt = ps.tile([H, P], F32, tag="pst")
        nc.tensor.transpose(pt[:], red[:], idt[:])
        rt = sb.tile([H, P], F32, tag="rt")
        nc.scalar.activation(rt[:], pt[:],
                             mybir.ActivationFunctionType.Copy)
        p2 = ps.tile([P, H], F32, tag="ps2")
        nc.tensor.matmul(p2[:], lhsT=rt[:], rhs=w2pt[:],
                         start=True, stop=True)
        h2ff = sb.tile([P, H], F32, tag="h2ff")
        nc.vector.tensor_add(h2ff[:], p2[:], c2t[:])
        nc.vector.tensor_scalar_mul(h2ff[:], h2ff[:], dst_[:, t:t + 1])
        nc.sync.dma_start(h2f[t * P:(t + 1) * P, :], h2ff[:])
        h2t = sb.tile([P, H], TD, tag="h2t")
        nc.vector.tensor_copy(h2t[:], h2ff[:])
        nc.sync.dma_start(h2s[t * P:(t + 1) * P, :], h2t[:])

    # ---- AllGather 2 ----
    if "ag2" in phases:
        nc.gpsimd.collective_compute(
            "AllGather", mybir.AluOpType.bypass,
            replica_groups=[list(range(C))],
            ins=[h2s.opt()], outs=[tab2.opt()])

    # ---- Layer 2 aggregation + epilogue (self via local f32 DMA) ----
    for t in range(T_real if "l2" in phases else 0):
        K = K_list[t]
        sf = sb.tile([P, H], F32, tag="sf2")
        nc.sync.dma_start(sf[:], h2f[t * P:(t + 1) * P, :])
        red = sb.tile([P, H], F32, tag="red2")
        if K > 0:
            g = gp.tile([P, maxK * H], TD, tag="g2")
            for j in range(K):
                nc.gpsimd.indirect_dma_start(
                    out=g[:, j * H:(j + 1) * H], out_offset=None,
                    in_=tab2[:],
                    in_offset=bass.IndirectOffsetOnAxis(
                        ap=ix2[:, offs[t] + j: offs[t] + j + 1], axis=0))
            nc.vector.reduce_sum(
                out=red[:],
                in_=g[:, :K * H].rearrange("p (j f) -> p f j", f=H),
                axis=mybir.AxisListType.X)
            nc.vector.tensor_add(red[:], red[:], sf[:])
        else:
            nc.vector.tensor_copy(red[:], sf[:])
        nc.vector.tensor_scalar_mul(red[:], red[:], dst_[:, t:t + 1])
        nc.vector.tensor_add(red[:], red[:], b2t[:])
        nc.vector.tensor_scalar_max(red[:], red[:], 0.0)
        ot = sb.tile([P, H], F32, tag="ot")
        nc.vector.tensor_copy(ot[:], red[:])
        nc.sync.dma_start(out[t * P:(t + 1) * P, :], ot[:])


def _impl(x, edge_index, W1, b1, W2, b2, gamma, beta, run_mean, run_var,
          n_nodes):
    x = np.asarray(x, np.float32)
    W1 = np.asarray(W1, np.float32)
    b1 = np.asarray(b1, np.float32)
    W2 = np.asarray(W2, np.float32)
    b2 = np.asarray(b2, np.float32)
    gamma = np.asarray(gamma, np.float32)
    beta = np.asarray(beta, np.float32)
    run_mean = np.asarray(run_mean, np.float32)
    run_var = np.asarray(run_var, np.float32)

    idx, dinv_s, meta = _plan(n_nodes, np.asarray(edge_index))
    per, SH = meta["per"], meta["SH"]

    # BN folding
    s = gamma / np.sqrt(run_var + BN_EPS)
    t = beta - run_mean * s
    W2p = (W2 * s[:, None]).astype(np.float32)
    c2 = (t @ W2).astype(np.float32)

    b1rep = np.tile(b1[None, :], (P, 1)).astype(np.float32)
    b2rep = np.tile(b2[None, :], (P, 1)).astype(np.float32)
    c2rep = np.tile(c2[None, :], (P, 1)).astype(np.float32)
    identv = np.eye(P, dtype=np.float32)

    nodes_by_cp = meta["nodes_by_cp"]
    in_maps = []
    for c in range(C):
        import ml_dtypes
        xs = np.zeros((SH, D), np.float32)
        xs[:per] = x[nodes_by_cp[c]]
        in_maps.append({
            "xT": np.ascontiguousarray(xs.T.astype(ml_dtypes.bfloat16)),
            "w1": W1, "w2p": W2p,
            "b1r": b1rep, "b2r": b2rep, "c2r": c2rep, "ident": identv,
            "dinvs": np.ascontiguousarray(dinv_s[c]),
            "idx1": np.ascontiguousarray(idx[c]),
        })

    nc = _build_nc(n_nodes, meta, tab_bf16=True)
    global _LAST_NC, _LAST_IN_MAPS, _LAST_META
    _LAST_NC, _LAST_IN_MAPS, _LAST_META = nc, in_maps, meta
    res = run_bass_kernel_spmd(nc, in_maps, core_ids=list(range(C))).results

    outf = np.zeros((n_nodes, H), np.float32)
    for c in range(C):
        outf[nodes_by_cp[c]] = res[c]["out"][:per]
    return outf


def kernel(x, edge_index, W1, b1, W2, b2, gamma, beta, run_mean, run_var):
    return _impl(x, edge_index, W1, b1, W2, b2, gamma, beta, run_mean,
                 run_var, n_nodes=100000)
